# revision 1
# baseline (speedup 1.0000x reference)
"""Trainium2 Bass kernel for nn_CrossAttentionExpert.

Problem (hardcoded shapes): B=4, C=256, H=W=64 (N=4096), C8=32.
  cross_p2v = attn(q=wq_p@f_p, k=wk_v@f_v, v=wv_v@f_v)
  cross_v2p = attn(q=wq_v@f_v, k=wk_p@f_p, v=wv_p@f_p)
  out = BN(w_out @ concat([f_p, f_v, cross_p2v, cross_v2p]))  (training-mode BN)

Sharding: 8 cores = (batch b, spatial half h).  Each core computes both
attention directions for its 2048 query positions (keys/values span all
4096 positions of its batch), the fused 1x1 output conv, and BN with a
[128,4] fp32 AllReduce of per-channel sum/sumsq across all 8 cores.

Key layout trick: scores are computed transposed, S^T[n,m] (n=key on
partitions, m=query on free axis) so that the exp'd probabilities can be
used directly as the moving operand of the V^T matmul — no transposes
anywhere on-chip.  All weight transposes are done host-side in numpy.
Softmax skips the max-subtraction (logits are O(25), exp fits fp32 with
huge margin for this problem's 0.05-scaled weights) and the 1/rowsum is
applied after the V-matmul via a PE outer-product broadcast.

All big matmuls run in float32r (PE fast-fp32, 4x throughput at N>=512);
f32r requires dst partition offset 0 and no tile_position, hence the
[32, *] Q/K layouts.  HW pitfalls found by bisect: tensor_tensor_reduce
(dual-output DVE) and activation-with-bias-AP crash the device — use
mul+reduce_sum and tensor_scalar instead.
"""

import numpy as np

import concourse.bass as bass
import concourse.mybir as mybir
import concourse.tile as tile
from concourse import bacc, bass_utils

FP = mybir.dt.float32
FR = mybir.dt.float32r  # PE fast-fp32 mode, 4x matmul throughput at N>=256
P = 128
C = 256
C8 = 32
N = 4096          # full spatial positions per batch
M = 2048          # local query positions per core
NMT = 4           # m-tiles of 512
MT = 512
NCORES = 8
BN_EPS = 1e-5
BN_COUNT = 4 * 4096  # B * H * W

_ALU = mybir.AluOpType
_ACT = mybir.ActivationFunctionType

_PROGRAM = None

# Debug bisect switches (set before first _get_program() call).
DBG_SKIP_ATTN = False       # skip attention (direct conv + BN only)
DBG_SKIP_COLLECTIVE = False  # use local stats instead of AllReduce
DBG_LEVEL = 99  # 1: loads+collective+writeback, 2: +direct conv, 3+: +BN math


def _build_program():
    nc = bacc.Bacc("TRN2", target_bir_lowering=False, debug=False,
                   num_devices=NCORES)

    # ---- DRAM I/O ----
    kv = [nc.dram_tensor(f"kv{d}", [C, N], FR, kind="ExternalInput").ap()
          for d in range(2)]
    wq = [nc.dram_tensor(f"wq{d}", [C, C8], FR, kind="ExternalInput").ap()
          for d in range(2)]
    wk = [nc.dram_tensor(f"wk{d}", [C, C8], FR, kind="ExternalInput").ap()
          for d in range(2)]
    wv = [nc.dram_tensor(f"wv{d}", [C, C], FR, kind="ExternalInput").ap()
          for d in range(2)]
    wout = nc.dram_tensor("wout", [4 * C, C], FR, kind="ExternalInput").ap()
    woutc = nc.dram_tensor("woutc", [4 * C, C], FP, kind="ExternalInput").ap()
    biasq = nc.dram_tensor("biasq", [P, 4], FP, kind="ExternalInput").ap()
    cvec = nc.dram_tensor("cvec", [P, 8], FP, kind="ExternalInput").ap()
    yout = nc.dram_tensor("y", [C, M], FP, kind="ExternalOutput").ap()

    with tile.TileContext(nc) as tc:
        with (
            tc.tile_pool(name="consts", bufs=1) as consts,
            tc.tile_pool(name="big", bufs=1) as big,
            tc.tile_pool(name="vt", bufs=32) as vtp,
            tc.tile_pool(name="st", bufs=1) as stp,
            tc.tile_pool(name="racc", bufs=1) as p_racc,
            tc.tile_pool(name="rp", bufs=1) as p_rp,
            tc.tile_pool(name="rbc", bufs=1) as p_rbc,
            tc.tile_pool(name="cross", bufs=2) as p_cross,
            tc.tile_pool(name="rinvp", bufs=1) as p_rinv,
            tc.tile_pool(name="small", bufs=4) as p_small,
            tc.tile_pool(name="psA", bufs=2, space="PSUM") as psA,
            tc.tile_pool(name="psB", bufs=2, space="PSUM") as psB,
            tc.tile_pool(name="psC", bufs=2, space="PSUM") as psC,
            tc.tile_pool(name="dram", bufs=1, space="DRAM") as dram,
        ):
            # ---- load constants / inputs to SBUF ----
            kv_sb = []
            for d in range(2):
                t = big.tile([P, 2, N], FR, name=f"kvsb{d}")
                src = kv[d].rearrange("(o p) n -> p o n", p=P)
                for o in range(2):
                    for q in range(4):
                        sl = slice(q * 1024, (q + 1) * 1024)
                        nc.sync.dma_start(t[:, o, sl], src[:, o, sl])
                kv_sb.append(t)

            def load_w(ap, shape, name, dt=FR):
                t = consts.tile(shape, dt, name=name)
                nc.sync.dma_start(
                    t[:], ap.rearrange("(o p) m -> p o m", p=P))
                return t

            wq_sb = [load_w(wq[d], [P, 2, C8], f"wqsb{d}") for d in range(2)]
            wk_sb = [load_w(wk[d], [P, 2, C8], f"wksb{d}") for d in range(2)]
            wv_sb = [load_w(wv[d], [P, 2, C], f"wvsb{d}") for d in range(2)]
            wout_sb = load_w(wout, [P, 8, C], "woutsb")
            woutc_sb = load_w(woutc, [P, 8, C], "woutcsb", dt=FP)
            biasq_sb = consts.tile([P, 4], FP, name="biasqsb")
            nc.sync.dma_start(biasq_sb[:], biasq[:])
            cvec_sb = consts.tile([P, 8], FP, name="cvecsb")
            nc.sync.dma_start(cvec_sb[:], cvec[:])

            ones_col = consts.tile([P, 1], FP, name="ones_col")
            nc.vector.memset(ones_col[:], 1.0)
            ones_row = consts.tile([1, P], FP, name="ones_row")
            nc.vector.memset(ones_row[:], 1.0)
            eps_t = consts.tile([P, 1], FP, name="eps_t")
            nc.vector.memset(eps_t[:], BN_EPS)

            # ---- persistent activations ----
            # qr[d]: Q result, C8 channels on partitions 0-31, [32, 2048]
            # kt[d]: K result, C8 on partitions 0-31, keys on free, [32, 4096]
            # (f32r matmuls require dst partition 0 / no tile_position)
            qr = [big.tile([32, M], FR, name=f"qr{d}") for d in range(2)]
            kt = [big.tile([32, N], FR, name=f"kt{d}") for d in range(2)]
            y_acc = [big.tile([P, M], FP, name=f"yacc{cc}") for cc in range(2)]

            # ---- direct terms of the output conv:
            #      y = wout[:, :256] @ f_p[:, half] + wout[:, 256:512] @ f_v[:, half]
            # f_p half = kv1[:, :2048]; f_v half = kv0[:, :2048].
            if DBG_LEVEL < 2:
                for cc in range(2):
                    nc.vector.memset(y_acc[cc][:], 0.5)
            for oc in range(2 if DBG_LEVEL >= 2 else 0):
                ocs = slice(oc * P, (oc + 1) * P)
                for t in range(NMT):
                    msl = slice(t * MT, (t + 1) * MT)
                    ps = psC.tile([P, MT], FP, tag="misc")
                    nc.tensor.matmul(ps, wout_sb[:, 0, ocs],
                                     kv_sb[1][:, 0, msl],
                                     start=True, stop=False)
                    nc.tensor.matmul(ps, wout_sb[:, 1, ocs],
                                     kv_sb[1][:, 1, msl],
                                     start=False, stop=False)
                    nc.tensor.matmul(ps, wout_sb[:, 2, ocs],
                                     kv_sb[0][:, 0, msl],
                                     start=False, stop=False)
                    nc.tensor.matmul(ps, wout_sb[:, 3, ocs],
                                     kv_sb[0][:, 1, msl],
                                     start=False, stop=True)
                    nc.scalar.copy(y_acc[oc][:, msl], ps)

            # ---- per-direction work ----
            for d in range(2 if not DBG_SKIP_ATTN else 0):
                qkv = kv_sb[1 - d]    # Q source (dir0: f_p=kv1, dir1: f_v=kv0)
                kkv = kv_sb[d]        # K/V source

                # Q conv: single [32, M] result, C8 channels on partitions 0-31.
                for t in range(NMT):
                    msl = slice(t * MT, (t + 1) * MT)
                    ps = psC.tile([32, MT], FP, tag="misc")
                    for kc in range(2):
                        nc.tensor.matmul(
                            ps, wq_sb[d][:, kc, :], qkv[:, kc, msl],
                            start=(kc == 0), stop=(kc == 1))
                    nc.scalar.activation(qr[d][:, msl], ps, _ACT.Identity,
                                         bias=biasq_sb[0:32, 2 * d:2 * d + 1])

                # K conv: [32, N], all 4096 keys along the free axis.
                for sub in range(8):
                    nsl = slice(sub * MT, (sub + 1) * MT)
                    ps = psC.tile([32, MT], FP, tag="misc")
                    for kc in range(2):
                        nc.tensor.matmul(
                            ps, wk_sb[d][:, kc, :], kkv[:, kc, nsl],
                            start=(kc == 0), stop=(kc == 1))
                    nc.scalar.activation(
                        kt[d][:, nsl], ps, _ACT.Identity,
                        bias=biasq_sb[0:32, 2 * d + 1:2 * d + 2])

                # V^T conv: vt[j] = f_kv[:, j*128:(j+1)*128]^T @ wv^T, [128, 256]
                vt_d = []
                for j in range(32):
                    ps = psC.tile([P, C], FP, tag="misc")
                    for kc in range(2):
                        nc.tensor.matmul(
                            ps, kkv[:, kc, j * P:(j + 1) * P],
                            wv_sb[d][:, kc, :],
                            start=(kc == 0), stop=(kc == 1))
                    v = vtp.tile([P, C], FR, tag="vt")
                    nc.scalar.copy(v[:], ps)
                    vt_d.append(v)

                # ---- attention over m-tiles ----
                for t in range(NMT):
                    msl = slice(t * MT, (t + 1) * MT)
                    av = [psB.tile([P, MT], FP, tag="av", name=f"av{i}")
                          for i in range(2)]
                    racc = p_racc.tile([P, MT], FP, tag="racc")
                    for burst in range(8):
                        bsl = slice(burst * P, (burst + 1) * P)
                        stg = stp.tile([P, 4 * MT], FR, tag="st")
                        for half in range(2):
                            pt = psA.tile([P, 2, MT], FP, tag="stps")
                            for rr in range(2):
                                rg = 2 * half + rr
                                ksl = slice(rg * 1024 + burst * P,
                                            rg * 1024 + (burst + 1) * P)
                                nc.tensor.matmul(
                                    pt[:, rr, :], kt[d][:, ksl],
                                    qr[d][:, msl],
                                    start=True, stop=True)
                            nc.scalar.activation(
                                stg[:, half * 1024:(half + 1) * 1024],
                                pt[:, :, :], _ACT.Exp)
                        # rowsum partials (sum over the 4 key-chunks here)
                        view = stg[:].rearrange("p (r m) -> p m r", m=MT)
                        if burst == 0:
                            nc.vector.reduce_sum(racc[:], view,
                                                 axis=mybir.AxisListType.X)
                        else:
                            rp = p_rp.tile([P, MT], FP, tag="rp")
                            nc.vector.reduce_sum(rp[:], view,
                                                 axis=mybir.AxisListType.X)
                            nc.vector.tensor_add(racc[:], racc[:], rp[:])
                        # V^T @ P accumulation
                        for rg in range(4):
                            j = rg * 8 + burst
                            ssl = slice(rg * MT, (rg + 1) * MT)
                            for cc in range(2):
                                nc.tensor.matmul(
                                    av[cc], vt_d[j][:, cc * P:(cc + 1) * P],
                                    stg[:, ssl],
                                    start=(burst == 0 and rg == 0),
                                    stop=(burst == 7 and rg == 3))
                    # 1/rowsum, broadcast to 128 partitions via outer product
                    rsum_ps = psC.tile([1, MT], FP, tag="misc")
                    nc.tensor.matmul(rsum_ps, ones_col[:], racc[:],
                                     start=True, stop=True)
                    rinv = p_rinv.tile([1, MT], FP, tag="rinv")
                    nc.vector.reciprocal(rinv[:], rsum_ps)
                    rbc_ps = psC.tile([P, MT], FP, tag="misc")
                    nc.tensor.matmul(rbc_ps, ones_row[:], rinv[:],
                                     start=True, stop=True)
                    rbc = p_rbc.tile([P, MT], FP, tag="rbc")
                    nc.vector.tensor_copy(rbc[:], rbc_ps)
                    # cross = av * (1/rowsum) + bv ; then y += wout_cross @ cross
                    crs = []
                    for cc in range(2):
                        cross = p_cross.tile([P, MT], FP, tag="cross")
                        nc.vector.tensor_mul(cross[:], av[cc], rbc[:])
                        nc.vector.tensor_scalar_add(
                            cross[:], cross[:],
                            cvec_sb[:, 2 * d + cc:2 * d + cc + 1])
                        crs.append(cross)
                    for oc in range(2):
                        ocs = slice(oc * P, (oc + 1) * P)
                        yc = psC.tile([P, MT], FP, tag="misc")
                        nc.tensor.matmul(yc, woutc_sb[:, 4 + 2 * d, ocs],
                                         crs[0][:], start=True, stop=False)
                        nc.tensor.matmul(yc, woutc_sb[:, 5 + 2 * d, ocs],
                                         crs[1][:], start=False, stop=True)
                        nc.vector.tensor_add(y_acc[oc][:, msl],
                                             y_acc[oc][:, msl], yc)

            # ---- BN: local stats, AllReduce, normalize ----
            stats = p_small.tile([P, 4], FP, tag="stats")
            if DBG_LEVEL < 3:
                nc.vector.memset(stats[:], 1.0)
            for cc in range(2 if DBG_LEVEL >= 3 else 0):
                nc.vector.reduce_sum(stats[:, cc:cc + 1], y_acc[cc][:],
                                     axis=mybir.AxisListType.X)
                scratch = stp.tile([P, 4 * MT], FP, tag="st")
                nc.vector.tensor_mul(scratch[:], y_acc[cc][:], y_acc[cc][:])
                nc.vector.reduce_sum(stats[:, 2 + cc:3 + cc], scratch[:],
                                     axis=mybir.AxisListType.X)
            cc_in = dram.tile([P, 4], FP)
            cc_out = dram.tile([P, 4], FP)
            nc.sync.dma_start(cc_in[:], stats[:])
            if DBG_SKIP_COLLECTIVE:
                nc.sync.dma_start(cc_out[:], cc_in[:])
            else:
                nc.gpsimd.collective_compute(
                    "AllReduce", _ALU.add,
                    replica_groups=[list(range(NCORES))],
                    ins=[cc_in.opt()], outs=[cc_out.opt()])
            ar = p_small.tile([P, 4], FP, tag="ar")
            nc.sync.dma_start(ar[:], cc_out[:])

            inv_n = 1.0 / BN_COUNT
            yo = yout.rearrange("(o p) m -> p o m", p=P)
            for cc in range(2):
                if DBG_LEVEL >= 3:
                    mean = p_small.tile([P, 1], FP, tag="bn")
                    ex2 = p_small.tile([P, 1], FP, tag="bn")
                    var = p_small.tile([P, 1], FP, tag="bn")
                    nc.vector.tensor_scalar_mul(mean[:], ar[:, cc:cc + 1],
                                                inv_n)
                    nc.vector.tensor_scalar_mul(ex2[:], ar[:, 2 + cc:3 + cc],
                                                inv_n)
                    nc.vector.tensor_tensor(var[:], mean[:], mean[:],
                                            _ALU.mult)
                    nc.vector.tensor_sub(var[:], ex2[:], var[:])
                    sd = p_small.tile([P, 1], FP, tag="bn")
                    nc.vector.tensor_scalar_add(var[:], var[:], BN_EPS)
                    nc.scalar.activation(sd[:], var[:], _ACT.Sqrt)
                    rstd = p_small.tile([P, 1], FP, tag="bn")
                    nc.vector.reciprocal(rstd[:], sd[:])
                    scale = p_small.tile([P, 1], FP, tag="bn")
                    nc.vector.tensor_tensor(scale[:],
                                            cvec_sb[:, 4 + cc:5 + cc],
                                            rstd[:], _ALU.mult)
                    shift = p_small.tile([P, 1], FP, tag="bn")
                    nc.vector.tensor_tensor(shift[:], mean[:], scale[:],
                                            _ALU.mult)
                    nc.vector.tensor_sub(shift[:], cvec_sb[:, 6 + cc:7 + cc],
                                         shift[:])
                    nc.vector.tensor_scalar(
                        out=y_acc[cc][:], in0=y_acc[cc][:],
                        scalar1=scale[:], scalar2=shift[:],
                        op0=_ALU.mult, op1=_ALU.add)
                for q in range(2):
                    qsl = slice(q * 1024, (q + 1) * 1024)
                    nc.sync.dma_start(yo[:, cc, qsl], y_acc[cc][:, qsl])

    nc.compile()
    return nc


def _get_program():
    global _PROGRAM
    if _PROGRAM is None:
        _PROGRAM = _build_program()
    return _PROGRAM


def _make_in_maps(inputs):
    f_p = np.ascontiguousarray(
        np.asarray(inputs["f_p"], np.float32).reshape(4, C, N))
    f_v = np.ascontiguousarray(
        np.asarray(inputs["f_v"], np.float32).reshape(4, C, N))

    def T(x):
        return np.ascontiguousarray(np.asarray(x, np.float32).T)

    # direction 0 (p2v): q from f_p, k/v from f_v; dir 1 (v2p): reversed.
    shared = {
        "wq0": T(inputs["wq_p"]), "wk0": T(inputs["wk_v"]),
        "wv0": T(inputs["wv_v"]),
        "wq1": T(inputs["wq_v"]), "wk1": T(inputs["wk_p"]),
        "wv1": T(inputs["wv_p"]),
        "wout": T(inputs["w_out"]),
        "woutc": T(inputs["w_out"]),
        "biasq": np.ascontiguousarray(np.stack(
            [np.tile(np.asarray(inputs[k], np.float32), 4)
             for k in ("bq_p", "bk_v", "bq_v", "bk_p")], axis=1)),
        "cvec": np.ascontiguousarray(np.stack(
            [np.asarray(inputs["bv_v"], np.float32)[:P],
             np.asarray(inputs["bv_v"], np.float32)[P:],
             np.asarray(inputs["bv_p"], np.float32)[:P],
             np.asarray(inputs["bv_p"], np.float32)[P:],
             np.asarray(inputs["gamma"], np.float32)[:P],
             np.asarray(inputs["gamma"], np.float32)[P:],
             np.asarray(inputs["beta"], np.float32)[:P],
             np.asarray(inputs["beta"], np.float32)[P:]], axis=1)),
    }
    in_maps = []
    for core in range(NCORES):
        b, h = divmod(core, 2)
        # roll so this core's query half sits at columns [0, 2048); K/V use
        # the full (permuted) range — softmax/AV are order-invariant in keys.
        kv1 = np.ascontiguousarray(np.roll(f_p[b], -h * M, axis=1))
        kv0 = np.ascontiguousarray(np.roll(f_v[b], -h * M, axis=1))
        in_maps.append({"kv0": kv0, "kv1": kv1, **shared})
    return in_maps


def _assemble(results):
    out = np.empty((4, C, N), np.float32)
    for core in range(NCORES):
        b, h = divmod(core, 2)
        out[b][:, h * M:(h + 1) * M] = results[core]["y"]
    return out.reshape(4, C, 64, 64)


def _run(inputs, **kwargs):
    nc = _get_program()
    in_maps = _make_in_maps(inputs)
    res = bass_utils.run_bass_kernel_spmd(
        nc, in_maps, core_ids=list(range(NCORES)), **kwargs)
    return _assemble(res.results), res


def kernel(**inputs):
    out, _ = _run(inputs)
    return out



# revision 8
# speedup vs baseline: 1.3901x; 1.3901x over previous
"""Trainium2 Bass kernel for nn_CrossAttentionExpert.

Problem (hardcoded shapes): B=4, C=256, H=W=64 (N=4096), C8=32.
  cross_p2v = attn(q=wq_p@f_p, k=wk_v@f_v, v=wv_v@f_v)
  cross_v2p = attn(q=wq_v@f_v, k=wk_p@f_p, v=wv_p@f_p)
  out = BN(w_out @ concat([f_p, f_v, cross_p2v, cross_v2p]))  (training BN)

Sharding: 8 cores = (batch b, spatial half h).  Each core computes both
attention directions for its 2048 query positions (keys/values span all
4096 positions of its batch), the fused 1x1 output conv, and BN with a
[128,4] fp32 AllReduce of per-channel sum/sumsq across all 8 cores.

Layout: scores are computed transposed, S^T[n,m] (n=key on partitions,
m=query on free axis) so the exp'd probabilities feed the V^T matmul
moving operand directly -- no on-chip transposes.  Softmax skips the
max-subtraction (logits are O(25); exp fits fp32/bf16 range with huge
margin for this problem's 0.05-scaled weights); 1/rowsum is applied
after the V-matmul via a PE outer-product broadcast.

Perf notes (v2):
 - Attention operands (qr/kt/stg/vt/cross) are bf16: same 1 cycle/row as
   f32r on PE but 2x faster LDWEIGHTS and half the SBUF. Measured rel
   err ~4e-3 vs the 2e-2 gate.
 - Rowsum partials run on the otherwise-idle Pool engine (gpsimd) as
   contiguous pairwise adds; the DVE strided reduce was 3.6us/burst.
 - Directions interleave per m-tile so the whole output conv (direct +
   both cross terms) accumulates in one PSUM group per (oc, m-tile) and
   y_acc is written exactly once (Identity+bias folds the wv biases).
 - DMA is ordered position-major and convs are emitted in data-arrival
   order so PE starts ~2.8us after launch.
 - fp32 tiles are bitcast to f32r for the rowsum-collapse/broadcast
   matmuls (f32r >=256-wide streams 1 cycle/row; fp32 is 4).
HW pitfalls (from bisect on the v1 kernel): tensor_tensor_reduce
(dual-output DVE) and f32r matmuls with nonzero dst partition offset
crash the device.
"""

import numpy as np

import concourse.bass as bass
import concourse.mybir as mybir
import concourse.tile as tile
from concourse import bacc, bass_utils

FP = mybir.dt.float32
FR = mybir.dt.float32r
BF = mybir.dt.bfloat16
P = 128
C = 256
C8 = 32
N = 4096          # full spatial positions per batch
M = 2048          # local query positions per core
NMT = 4           # m-tiles of 512
MT = 512
NCORES = 8
BN_EPS = 1e-5
BN_COUNT = 4 * 4096  # B * H * W

_ALU = mybir.AluOpType
_ACT = mybir.ActivationFunctionType

_PROGRAM = None


def _build_program():
    nc = bacc.Bacc("TRN2", target_bir_lowering=False, debug=False,
                   num_devices=NCORES)

    # ---- DRAM I/O ----
    kv = [nc.dram_tensor(f"kv{d}", [C, N], FR, kind="ExternalInput").ap()
          for d in range(2)]
    wq = [nc.dram_tensor(f"wq{d}", [C, C8], FR, kind="ExternalInput").ap()
          for d in range(2)]
    wk = [nc.dram_tensor(f"wk{d}", [C, C8], FR, kind="ExternalInput").ap()
          for d in range(2)]
    wv = [nc.dram_tensor(f"wv{d}", [C, C], FR, kind="ExternalInput").ap()
          for d in range(2)]
    wout = nc.dram_tensor("wout", [2 * C, C], FR, kind="ExternalInput").ap()
    woutc = nc.dram_tensor("woutc", [2 * C, C], BF, kind="ExternalInput").ap()
    biasq = nc.dram_tensor("biasq", [P, 4], FP, kind="ExternalInput").ap()
    ybias = nc.dram_tensor("ybias", [P, 2], FP, kind="ExternalInput").ap()
    gb = nc.dram_tensor("gb", [P, 4], FP, kind="ExternalInput").ap()
    yout = nc.dram_tensor("y", [C, M], FP, kind="ExternalOutput").ap()

    with tile.TileContext(nc) as tc:
        with (
            nc.allow_low_precision(
                reason="f32r/bf16 attention intermediates; "
                       "end-to-end rel err ~4e-3 vs 2e-2 gate"),
            tc.tile_pool(name="consts", bufs=1) as consts,
            tc.tile_pool(name="big", bufs=1) as big,
            tc.tile_pool(name="vt", bufs=64) as vtp,
            tc.tile_pool(name="st", bufs=2) as stp,
            tc.tile_pool(name="rs", bufs=2) as rsp,
            tc.tile_pool(name="cross", bufs=4) as p_cross,
            tc.tile_pool(name="small", bufs=4) as p_small,
            tc.tile_pool(name="psA", bufs=2, space="PSUM") as psA,
            tc.tile_pool(name="psB", bufs=1, space="PSUM") as psB,
            tc.tile_pool(name="psC", bufs=2, space="PSUM") as psC,
            tc.tile_pool(name="dram", bufs=1, space="DRAM") as dram,
        ):
            # ---- small constants first (cheap DMAs) ----
            def load_w(ap, shape, name, dt=FR):
                t = consts.tile(shape, dt, name=name)
                nc.sync.dma_start(
                    t[:], ap.rearrange("(o p) m -> p o m", p=P))
                return t

            wq_sb = [load_w(wq[d], [P, 2, C8], f"wqsb{d}") for d in range(2)]
            wk_sb = [load_w(wk[d], [P, 2, C8], f"wksb{d}") for d in range(2)]
            wv_sb = [load_w(wv[d], [P, 2, C], f"wvsb{d}") for d in range(2)]
            wout_sb = load_w(wout, [P, 4, C], "woutsb")
            woutc_sb = load_w(woutc, [P, 4, C], "woutcsb", dt=BF)
            biasq_sb = consts.tile([P, 4], FP, name="biasqsb")
            nc.sync.dma_start(biasq_sb[:], biasq[:])
            ybias_sb = consts.tile([P, 2], FP, name="ybiassb")
            nc.sync.dma_start(ybias_sb[:], ybias[:])
            gb_sb = consts.tile([P, 4], FP, name="gbsb")
            nc.sync.dma_start(gb_sb[:], gb[:])

            ones_col = consts.tile([P, 1], BF, name="ones_col")
            nc.vector.memset(ones_col[:], 1.0)
            ones_row = consts.tile([1, P], BF, name="ones_row")
            nc.vector.memset(ones_row[:], 1.0)

            # ---- kv loads, position-major so convs can start early ----
            kv_sb = []
            for d in range(2):
                t = big.tile([P, 2, N], FR, name=f"kvsb{d}")
                kv_sb.append(t)
            for d in range(2):
                src = kv[d].rearrange("(o p) n -> p o n", p=P)
                for q in range(4):
                    sl = slice(q * 1024, (q + 1) * 1024)
                    for o in range(2):
                        nc.sync.dma_start(kv_sb[d][:, o, sl], src[:, o, sl])

            # ---- persistent activations ----
            qr = [big.tile([32, M], BF, name=f"qr{d}") for d in range(2)]
            kt = [big.tile([32, N], BF, name=f"kt{d}") for d in range(2)]
            y_acc = [big.tile([P, M], FP, name=f"yacc{cc}") for cc in range(2)]

            # ---- projections, in DMA-arrival order ----
            # dir0: q from kv1 (f_p), k/v from kv0 (f_v); dir1 swapped.
            def k_conv(d):
                kkv = kv_sb[d]
                for sub in range(8):
                    nsl = slice(sub * MT, (sub + 1) * MT)
                    ps = psC.tile([32, MT], FP, tag="misc", name="kps")
                    for kc in range(2):
                        nc.tensor.matmul(
                            ps, wk_sb[d][:, kc, :], kkv[:, kc, nsl],
                            start=(kc == 0), stop=(kc == 1))
                    nc.scalar.activation(
                        kt[d][:, nsl], ps, _ACT.Identity,
                        bias=biasq_sb[0:32, 2 * d + 1:2 * d + 2])

            def v_conv(d):
                kkv = kv_sb[d]
                vt_d = []
                for j in range(32):
                    ps = psC.tile([P, C], FP, tag="misc", name="vps")
                    for kc in range(2):
                        nc.tensor.matmul(
                            ps, kkv[:, kc, j * P:(j + 1) * P],
                            wv_sb[d][:, kc, :],
                            start=(kc == 0), stop=(kc == 1))
                    v = vtp.tile([P, C], BF, tag="vt", name="vtt")
                    nc.vector.tensor_copy(v[:], ps)
                    vt_d.append(v)
                return vt_d

            def q_conv(d):
                qkv = kv_sb[1 - d]
                for t in range(NMT):
                    msl = slice(t * MT, (t + 1) * MT)
                    ps = psC.tile([32, MT], FP, tag="misc", name="qps")
                    for kc in range(2):
                        nc.tensor.matmul(
                            ps, wq_sb[d][:, kc, :], qkv[:, kc, msl],
                            start=(kc == 0), stop=(kc == 1))
                    nc.scalar.activation(qr[d][:, msl], ps, _ACT.Identity,
                                         bias=biasq_sb[0:32, 2 * d:2 * d + 1])

            k_conv(0)
            vt = [None, None]
            vt[0] = v_conv(0)
            q_conv(1)          # reads kv0[:, :, :2048] -- already resident
            q_conv(0)          # reads kv1[:, :, :2048]
            k_conv(1)
            vt[1] = v_conv(1)

            # ---- attention + fused output conv, per m-tile ----
            for t in range(NMT):
                msl = slice(t * MT, (t + 1) * MT)
                crs = [[None, None], [None, None]]
                for d in range(2):
                    av = [psB.tile([P, MT], FP, tag=f"av{i}", name=f"av{i}")
                          for i in range(2)]
                    racc = rsp.tile([P, MT], FP, tag="racc", name="racc")
                    racc_bf = rsp.tile([P, MT], BF, tag="raccbf",
                                       name="racc_bf")
                    for burst in range(8):
                        stg = stp.tile([P, 4 * MT], BF, tag="st", name="stg")
                        for half in range(2):
                            pt = psA.tile([P, 2, MT], FP, tag="pt", name="pt")
                            for rr in range(2):
                                rg = 2 * half + rr
                                ksl = slice(rg * 1024 + burst * P,
                                            rg * 1024 + (burst + 1) * P)
                                nc.tensor.matmul(
                                    pt[:, rr, :], kt[d][:, ksl],
                                    qr[d][:, msl],
                                    start=True, stop=True)
                            nc.scalar.activation(
                                stg[:, half * 1024:(half + 1) * 1024],
                                pt[:, :, :], _ACT.Exp)
                        # rowsum partials on Pool: pairwise adds, fp32 accum
                        t1 = rsp.tile([P, 2 * MT], FP, tag="t1", name="t1")
                        nc.gpsimd.tensor_add(t1[:], stg[:, 0:1024],
                                             stg[:, 1024:2048])
                        if burst == 0:
                            nc.gpsimd.tensor_add(racc[:], t1[:, 0:MT],
                                                 t1[:, MT:2 * MT])
                        else:
                            t2 = rsp.tile([P, MT], FP, tag="t2", name="t2")
                            nc.gpsimd.tensor_add(t2[:], t1[:, 0:MT],
                                                 t1[:, MT:2 * MT])
                            nc.gpsimd.tensor_add(
                                racc_bf[:] if burst == 7 else racc[:],
                                racc[:], t2[:])
                        # V^T @ P accumulation
                        for rg in range(4):
                            j = rg * 8 + burst
                            ssl = slice(rg * MT, (rg + 1) * MT)
                            for cc in range(2):
                                nc.tensor.matmul(
                                    av[cc], vt[d][j][:, cc * P:(cc + 1) * P],
                                    stg[:, ssl],
                                    start=(burst == 0 and rg == 0),
                                    stop=(burst == 7 and rg == 3))
                    # collapse rowsum across partitions; broadcast 1/rowsum
                    rsum_ps = psC.tile([1, MT], FP, tag="misc", name="rsum")
                    nc.tensor.matmul(rsum_ps, ones_col[:], racc_bf[:],
                                     start=True, stop=True)
                    rs_bf = p_small.tile([1, MT], BF, tag="rsbf", name="rsbf")
                    nc.vector.tensor_copy(rs_bf[:], rsum_ps)
                    rbc_ps = psC.tile([P, MT], FP, tag="misc", name="rbc")
                    nc.tensor.matmul(rbc_ps, ones_row[:], rs_bf[:],
                                     start=True, stop=True)
                    rbc = p_cross.tile([P, MT], FP, tag="rbc", name="rbc_sb")
                    nc.vector.reciprocal(rbc[:], rbc_ps)
                    for cc in range(2):
                        cross = p_cross.tile([P, MT], BF, tag="cross",
                                             name="cross")
                        nc.vector.tensor_mul(cross[:], av[cc], rbc[:])
                        crs[d][cc] = cross
                # fused output conv: direct + both cross terms, one PSUM
                # group per oc; y_acc written once with the wv-bias folded.
                for oc in range(2):
                    ocs = slice(oc * P, (oc + 1) * P)
                    yc = psC.tile([P, MT], FP, tag="misc", name="yc")
                    nc.tensor.matmul(yc, wout_sb[:, 0, ocs],
                                     kv_sb[1][:, 0, msl],
                                     start=True, stop=False)
                    nc.tensor.matmul(yc, wout_sb[:, 1, ocs],
                                     kv_sb[1][:, 1, msl],
                                     start=False, stop=False)
                    nc.tensor.matmul(yc, wout_sb[:, 2, ocs],
                                     kv_sb[0][:, 0, msl],
                                     start=False, stop=False)
                    nc.tensor.matmul(yc, wout_sb[:, 3, ocs],
                                     kv_sb[0][:, 1, msl],
                                     start=False, stop=False)
                    for d in range(2):
                        for cc in range(2):
                            nc.tensor.matmul(
                                yc, woutc_sb[:, 2 * d + cc, ocs],
                                crs[d][cc][:],
                                start=False,
                                stop=(d == 1 and cc == 1))
                    nc.scalar.activation(y_acc[oc][:, msl], yc, _ACT.Identity,
                                         bias=ybias_sb[:, oc:oc + 1])

            # ---- BN: local stats, AllReduce, normalize ----
            stats = p_small.tile([P, 4], FP, tag="stats", name="stats")
            sqacc = p_small.tile([P, 4], FP, tag="sqacc", name="sqacc")
            for cc in range(2):
                nc.vector.reduce_sum(stats[:, cc:cc + 1], y_acc[cc][:],
                                     axis=mybir.AxisListType.X)
                for hh in range(2):
                    sq = rsp.tile([P, 2 * MT], FP, tag="t1", name="sq")
                    nc.scalar.activation(
                        sq[:], y_acc[cc][:, hh * 1024:(hh + 1) * 1024],
                        _ACT.Square,
                        accum_out=stats[:, 2 + cc:3 + cc] if hh == 0
                        else sqacc[:, 2 + cc:3 + cc])
                nc.gpsimd.tensor_add(stats[:, 2 + cc:3 + cc],
                                     stats[:, 2 + cc:3 + cc],
                                     sqacc[:, 2 + cc:3 + cc])
            cc_in = dram.tile([P, 4], FP, name="cc_in")
            cc_out = dram.tile([P, 4], FP, name="cc_out")
            nc.sync.dma_start(cc_in[:], stats[:])
            nc.gpsimd.collective_compute(
                "AllReduce", _ALU.add,
                replica_groups=[list(range(NCORES))],
                ins=[cc_in.opt()], outs=[cc_out.opt()])
            ar = p_small.tile([P, 4], FP, tag="ar", name="ar")
            nc.sync.dma_start(ar[:], cc_out[:])

            inv_n = 1.0 / BN_COUNT
            yo = yout.rearrange("(o p) m -> p o m", p=P)
            for cc in range(2):
                mean = p_small.tile([P, 1], FP, tag="bn", name="mean")
                ex2 = p_small.tile([P, 1], FP, tag="bn", name="ex2")
                var = p_small.tile([P, 1], FP, tag="bn", name="var")
                nc.vector.tensor_scalar_mul(mean[:], ar[:, cc:cc + 1], inv_n)
                nc.vector.tensor_scalar_mul(ex2[:], ar[:, 2 + cc:3 + cc],
                                            inv_n)
                nc.vector.tensor_tensor(var[:], mean[:], mean[:], _ALU.mult)
                nc.vector.tensor_sub(var[:], ex2[:], var[:])
                sd = p_small.tile([P, 1], FP, tag="bn", name="sd")
                nc.vector.tensor_scalar_add(var[:], var[:], BN_EPS)
                nc.scalar.activation(sd[:], var[:], _ACT.Sqrt)
                rstd = p_small.tile([P, 1], FP, tag="bn", name="rstd")
                nc.vector.reciprocal(rstd[:], sd[:])
                scale = p_small.tile([P, 1], FP, tag="bn", name="scale")
                nc.vector.tensor_tensor(scale[:], gb_sb[:, cc:cc + 1],
                                        rstd[:], _ALU.mult)
                shift = p_small.tile([P, 1], FP, tag="bn", name="shift")
                nc.vector.tensor_tensor(shift[:], mean[:], scale[:],
                                        _ALU.mult)
                nc.vector.tensor_sub(shift[:], gb_sb[:, 2 + cc:3 + cc],
                                     shift[:])
                nc.vector.tensor_scalar(
                    out=y_acc[cc][:], in0=y_acc[cc][:],
                    scalar1=scale[:], scalar2=shift[:],
                    op0=_ALU.mult, op1=_ALU.add)
                for q in range(2):
                    qsl = slice(q * 1024, (q + 1) * 1024)
                    nc.sync.dma_start(yo[:, cc, qsl], y_acc[cc][:, qsl])

    nc.compile()
    return nc


def _get_program():
    global _PROGRAM
    if _PROGRAM is None:
        _PROGRAM = _build_program()
    return _PROGRAM


def _make_in_maps(inputs):
    BF_NP = mybir.dt.np(mybir.dt.bfloat16)
    f_p = np.ascontiguousarray(
        np.asarray(inputs["f_p"], np.float32).reshape(4, C, N))
    f_v = np.ascontiguousarray(
        np.asarray(inputs["f_v"], np.float32).reshape(4, C, N))

    def T(x):
        return np.ascontiguousarray(np.asarray(x, np.float32).T)

    w_out = np.asarray(inputs["w_out"], np.float32)
    bv_v = np.asarray(inputs["bv_v"], np.float32)
    bv_p = np.asarray(inputs["bv_p"], np.float32)
    # wv-bias terms of the cross contributions, folded into one vector.
    yb = w_out[:, 2 * C:3 * C] @ bv_v + w_out[:, 3 * C:] @ bv_p
    shared = {
        "wq0": T(inputs["wq_p"]), "wk0": T(inputs["wk_v"]),
        "wv0": T(inputs["wv_v"]),
        "wq1": T(inputs["wq_v"]), "wk1": T(inputs["wk_p"]),
        "wv1": T(inputs["wv_p"]),
        "wout": T(w_out[:, :2 * C]),
        "woutc": np.ascontiguousarray(T(w_out[:, 2 * C:]).astype(BF_NP)),
        "biasq": np.ascontiguousarray(np.stack(
            [np.tile(np.asarray(inputs[k], np.float32), 4)
             for k in ("bq_p", "bk_v", "bq_v", "bk_p")], axis=1)),
        "ybias": np.ascontiguousarray(np.stack([yb[:P], yb[P:]], axis=1)),
        "gb": np.ascontiguousarray(np.stack(
            [np.asarray(inputs["gamma"], np.float32)[:P],
             np.asarray(inputs["gamma"], np.float32)[P:],
             np.asarray(inputs["beta"], np.float32)[:P],
             np.asarray(inputs["beta"], np.float32)[P:]], axis=1)),
    }
    in_maps = []
    for core in range(NCORES):
        b, h = divmod(core, 2)
        # roll so this core's query half sits at columns [0, 2048); K/V use
        # the full (permuted) range -- softmax/AV are key-order-invariant.
        kv1 = np.ascontiguousarray(np.roll(f_p[b], -h * M, axis=1))
        kv0 = np.ascontiguousarray(np.roll(f_v[b], -h * M, axis=1))
        in_maps.append({"kv0": kv0, "kv1": kv1, **shared})
    return in_maps


def _assemble(results):
    out = np.empty((4, C, N), np.float32)
    for core in range(NCORES):
        b, h = divmod(core, 2)
        out[b][:, h * M:(h + 1) * M] = results[core]["y"]
    return out.reshape(4, C, 64, 64)


def _run(inputs, **kwargs):
    nc = _get_program()
    in_maps = _make_in_maps(inputs)
    res = bass_utils.run_bass_kernel_spmd(
        nc, in_maps, core_ids=list(range(NCORES)), **kwargs)
    return _assemble(res.results), res


def kernel(**inputs):
    out, _ = _run(inputs)
    return out


# revision 9
# speedup vs baseline: 1.7169x; 1.2351x over previous
"""Trainium2 Bass kernel for nn_CrossAttentionExpert.

Problem (hardcoded shapes): B=4, C=256, H=W=64 (N=4096), C8=32.
  cross_p2v = attn(q=wq_p@f_p, k=wk_v@f_v, v=wv_v@f_v)
  cross_v2p = attn(q=wq_v@f_v, k=wk_p@f_p, v=wv_p@f_p)
  out = BN(w_out @ concat([f_p, f_v, cross_p2v, cross_v2p]))  (training BN)

Sharding: 8 cores = (batch b, spatial half h).  Each core computes both
attention directions for its 2048 query positions (keys/values span all
4096 positions of its batch), the fused 1x1 output conv, and BN with a
[128,4] fp32 AllReduce of per-channel sum/sumsq across all 8 cores.

Layout: scores are computed transposed, S^T[n,m] (n=key on partitions,
m=query on free axis) so the exp'd probabilities feed the V^T matmul
moving operand directly -- no on-chip transposes.  Softmax skips the
max-subtraction (logits are O(25); exp fits fp32/bf16 range with huge
margin for this problem's 0.05-scaled weights); 1/rowsum is applied
after the V-matmul via a PE outer-product broadcast.

Perf structure (v3):
 - Attention operands (qr/kt/stg/vt/cross) are bf16: 1 cycle/row on PE,
   2x faster LDWEIGHTS, half the SBUF.  End-to-end rel err ~4.5e-3 vs
   the 2e-2 gate (validated in numpy and on HW).
 - Software pipeline: AV matmuls of burst b-1 are emitted after the
   score matmuls of burst b, so the PE never head-of-line blocks on the
   ACT exp of the current burst.  Tail work (rowsum collapse, 1/rowsum,
   cross muls), the fused output conv, and BN stat partials of tile t
   are emitted inside tile t+1's burst stream for the same reason.
 - Rowsum: bf16 pairwise adds on DVE (t1/t2), fp32 burst accumulation
   on the Pool engine, final value rounded to bf16 so the collapse and
   broadcast matmuls run as 1-cycle/row bf16 (f32r can't: codegen
   rejects degenerate-stationary f32r matmuls).
 - PSUM budget (8 banks): pt 2x2 banks, av0 2, av1 1, misc 1.
 - Conv phase is interleaved with the position-major kv DMA stream, and
   a dummy warmup AllReduce hides the collective's first-use latency.
"""

import numpy as np

import concourse.bass as bass
import concourse.mybir as mybir
import concourse.tile as tile
from concourse import bacc, bass_utils

FP = mybir.dt.float32
FR = mybir.dt.float32r
BF = mybir.dt.bfloat16
P = 128
C = 256
C8 = 32
N = 4096          # full spatial positions per batch
M = 2048          # local query positions per core
NMT = 4           # m-tiles of 512
MT = 512
NCORES = 8
BN_EPS = 1e-5
BN_COUNT = 4 * 4096  # B * H * W

_ALU = mybir.AluOpType
_ACT = mybir.ActivationFunctionType

_PROGRAM = None


def _build_program():
    nc = bacc.Bacc("TRN2", target_bir_lowering=False, debug=False,
                   num_devices=NCORES)

    # ---- DRAM I/O ----
    kv = [nc.dram_tensor(f"kv{d}", [C, N], FR, kind="ExternalInput").ap()
          for d in range(2)]
    wq = [nc.dram_tensor(f"wq{d}", [C, C8], FR, kind="ExternalInput").ap()
          for d in range(2)]
    wk = [nc.dram_tensor(f"wk{d}", [C, C8], FR, kind="ExternalInput").ap()
          for d in range(2)]
    wv = [nc.dram_tensor(f"wv{d}", [C, C], FR, kind="ExternalInput").ap()
          for d in range(2)]
    wout = nc.dram_tensor("wout", [2 * C, C], FR, kind="ExternalInput").ap()
    woutc = nc.dram_tensor("woutc", [2 * C, C], BF, kind="ExternalInput").ap()
    biasq = nc.dram_tensor("biasq", [P, 4], FP, kind="ExternalInput").ap()
    ybias = nc.dram_tensor("ybias", [P, 2], FP, kind="ExternalInput").ap()
    gb = nc.dram_tensor("gb", [P, 4], FP, kind="ExternalInput").ap()
    yout = nc.dram_tensor("y", [C, M], FP, kind="ExternalOutput").ap()

    with tile.TileContext(nc) as tc:
        with (
            nc.allow_low_precision(
                reason="bf16 attention intermediates; "
                       "end-to-end rel err ~4.5e-3 vs 2e-2 gate"),
            tc.tile_pool(name="consts", bufs=1) as consts,
            tc.tile_pool(name="big", bufs=1) as big,
            tc.tile_pool(name="vt", bufs=64) as vtp,
            tc.tile_pool(name="st", bufs=2) as stp,
            tc.tile_pool(name="rs", bufs=2) as rsp,
            tc.tile_pool(name="cross", bufs=4) as p_cross,
            tc.tile_pool(name="small", bufs=4) as p_small,
            tc.tile_pool(name="psA", bufs=2, space="PSUM") as psA,
            tc.tile_pool(name="psB", bufs=1, space="PSUM") as psB,
            tc.tile_pool(name="psC", bufs=1, space="PSUM") as psC,
            tc.tile_pool(name="dram", bufs=1, space="DRAM") as dram,
        ):
            # ---- small constants first (cheap DMAs) ----
            def load_w(ap, shape, name, dt=FR):
                t = consts.tile(shape, dt, name=name)
                nc.sync.dma_start(
                    t[:], ap.rearrange("(o p) m -> p o m", p=P))
                return t

            wq_sb = [load_w(wq[d], [P, 2, C8], f"wqsb{d}") for d in range(2)]
            wk_sb = [load_w(wk[d], [P, 2, C8], f"wksb{d}") for d in range(2)]
            wv_sb = [load_w(wv[d], [P, 2, C], f"wvsb{d}") for d in range(2)]
            wout_sb = load_w(wout, [P, 4, C], "woutsb")
            woutc_sb = load_w(woutc, [P, 4, C], "woutcsb", dt=BF)
            biasq_sb = consts.tile([P, 4], FP, name="biasqsb")
            nc.sync.dma_start(biasq_sb[:], biasq[:])
            ybias_sb = consts.tile([P, 2], FP, name="ybiassb")
            nc.sync.dma_start(ybias_sb[:], ybias[:])
            gb_sb = consts.tile([P, 4], FP, name="gbsb")
            nc.sync.dma_start(gb_sb[:], gb[:])

            ones_col = consts.tile([P, 1], BF, name="ones_col")
            nc.vector.memset(ones_col[:], 1.0)
            ones_row = consts.tile([1, P], BF, name="ones_row")
            nc.vector.memset(ones_row[:], 1.0)

            # ---- kv loads, position-major so convs can start early ----
            kv_sb = [big.tile([P, 2, N], FR, name=f"kvsb{d}")
                     for d in range(2)]
            for d in range(2):
                src = kv[d].rearrange("(o p) n -> p o n", p=P)
                for q in range(4):
                    sl = slice(q * 1024, (q + 1) * 1024)
                    for o in range(2):
                        nc.sync.dma_start(kv_sb[d][:, o, sl], src[:, o, sl])

            # warm up the collective path while convs run; result unused.
            warm_in = dram.tile([P, 4], FP, name="warm_in")
            warm_out = dram.tile([P, 4], FP, name="warm_out")
            nc.gpsimd.collective_compute(
                "AllReduce", _ALU.add,
                replica_groups=[list(range(NCORES))],
                ins=[warm_in.opt()], outs=[warm_out.opt()])

            # ---- persistent activations ----
            qr = [big.tile([32, M], BF, name=f"qr{d}") for d in range(2)]
            kt = [big.tile([32, N], BF, name=f"kt{d}") for d in range(2)]
            y_acc = [big.tile([P, M], FP, name=f"yacc{cc}") for cc in range(2)]
            vt = [[], []]

            # ---- projections, in DMA-arrival order ----
            # dir0: q from kv1 (f_p), k/v from kv0 (f_v); dir1 swapped.
            def k_conv(d, subs):
                kkv = kv_sb[d]
                for sub in subs:
                    nsl = slice(sub * MT, (sub + 1) * MT)
                    ps = psA.tile([32, MT], FP, tag="pt", name="kps")
                    for kc in range(2):
                        nc.tensor.matmul(
                            ps, wk_sb[d][:, kc, :], kkv[:, kc, nsl],
                            start=(kc == 0), stop=(kc == 1))
                    nc.scalar.activation(
                        kt[d][:, nsl], ps, _ACT.Identity,
                        bias=biasq_sb[0:32, 2 * d + 1:2 * d + 2])

            def v_conv(d, js):
                kkv = kv_sb[d]
                for j in js:
                    ps = psA.tile([P, C], FP, tag="pt", name="vps")
                    for kc in range(2):
                        nc.tensor.matmul(
                            ps, kkv[:, kc, j * P:(j + 1) * P],
                            wv_sb[d][:, kc, :],
                            start=(kc == 0), stop=(kc == 1))
                    v = vtp.tile([P, C], BF, tag="vt", name="vtt")
                    nc.vector.tensor_copy(v[:], ps)
                    vt[d].append(v)

            def q_conv(d):
                qkv = kv_sb[1 - d]
                for t in range(NMT):
                    msl = slice(t * MT, (t + 1) * MT)
                    ps = psA.tile([32, MT], FP, tag="pt", name="qps")
                    for kc in range(2):
                        nc.tensor.matmul(
                            ps, wq_sb[d][:, kc, :], qkv[:, kc, msl],
                            start=(kc == 0), stop=(kc == 1))
                    nc.scalar.activation(qr[d][:, msl], ps, _ACT.Identity,
                                         bias=biasq_sb[0:32, 2 * d:2 * d + 1])

            for d in range(2):
                k_conv(d, [0, 1]); v_conv(d, range(0, 8))
                k_conv(d, [2, 3]); v_conv(d, range(8, 16))
                q_conv(1 - d)  # q source is kv[d]'s first half, now resident
                k_conv(d, [4, 5]); v_conv(d, range(16, 24))
                k_conv(d, [6, 7]); v_conv(d, range(24, 32))

            # ---- BN stat partials, accumulated per m-tile ----
            ssum = p_small.tile([P, 2, NMT], FP, tag="ssum", name="ssum")
            ssq = p_small.tile([P, 2, NMT], FP, tag="ssq", name="ssq")

            # ---- attention + fused output conv ----
            # Tail/yc/stats of tile t are emitted inside tile t+1's burst
            # stream so the in-order PE queue never blocks on them.
            crs_all = {}

            def make_tail(t, d, av, racc_bf):
                def emit():
                    rsum_ps = psC.tile([1, MT], FP, tag="misc", name="rsum")
                    nc.tensor.matmul(rsum_ps, ones_col[:], racc_bf[:],
                                     start=True, stop=True)
                    rs_bf = p_small.tile([1, MT], BF, tag="rsbf",
                                         name="rsbf")
                    nc.vector.tensor_copy(rs_bf[:], rsum_ps)
                    rbc_ps = psC.tile([P, MT], FP, tag="misc", name="rbc")
                    nc.tensor.matmul(rbc_ps, ones_row[:], rs_bf[:],
                                     start=True, stop=True)
                    rbc = p_cross.tile([P, MT], FP, tag="rbc", name="rbc_sb")
                    nc.vector.reciprocal_approx_fast(out=rbc[:], in_=rbc_ps)
                    for cc in range(2):
                        cross = p_cross.tile([P, MT], BF, tag="cross",
                                             name="cross")
                        nc.vector.tensor_mul(cross[:], av[cc], rbc[:])
                        crs_all[(t, d, cc)] = cross
                return emit

            def make_yc(t):
                msl = slice(t * MT, (t + 1) * MT)

                def emit():
                    for oc in range(2):
                        ocs = slice(oc * P, (oc + 1) * P)
                        yc = psC.tile([P, MT], FP, tag="misc", name="yc")
                        nc.tensor.matmul(yc, wout_sb[:, 0, ocs],
                                         kv_sb[1][:, 0, msl],
                                         start=True, stop=False)
                        nc.tensor.matmul(yc, wout_sb[:, 1, ocs],
                                         kv_sb[1][:, 1, msl],
                                         start=False, stop=False)
                        nc.tensor.matmul(yc, wout_sb[:, 2, ocs],
                                         kv_sb[0][:, 0, msl],
                                         start=False, stop=False)
                        nc.tensor.matmul(yc, wout_sb[:, 3, ocs],
                                         kv_sb[0][:, 1, msl],
                                         start=False, stop=False)
                        for d in range(2):
                            for cc in range(2):
                                nc.tensor.matmul(
                                    yc, woutc_sb[:, 2 * d + cc, ocs],
                                    crs_all[(t, d, cc)][:],
                                    start=False,
                                    stop=(d == 1 and cc == 1))
                        nc.scalar.activation(y_acc[oc][:, msl], yc,
                                             _ACT.Identity,
                                             bias=ybias_sb[:, oc:oc + 1])
                return emit

            def make_stats(t):
                msl = slice(t * MT, (t + 1) * MT)

                def emit():
                    for cc in range(2):
                        nc.vector.reduce_sum(ssum[:, cc, t:t + 1],
                                             y_acc[cc][:, msl],
                                             axis=mybir.AxisListType.X)
                        sq = p_small.tile([P, MT], BF, tag="sq", name="sq",
                                          bufs=2)
                        nc.vector.scalar_tensor_tensor(
                            out=sq[:], in0=y_acc[cc][:, msl], scalar=1.0,
                            in1=y_acc[cc][:, msl],
                            op0=_ALU.mult, op1=_ALU.mult,
                            accum_out=ssq[:, cc, t:t + 1])
                return emit

            pend_tail = pend_yc = pend_stats = None
            for t in range(NMT):
                msl = slice(t * MT, (t + 1) * MT)
                for d in range(2):
                    av = [psB.tile([P, MT], FP, tag=f"av{i}", name=f"av{i}",
                                   bufs=2 - i) for i in range(2)]
                    racc = rsp.tile([P, MT], FP, tag="racc", name="racc")
                    racc_bf = rsp.tile([P, MT], BF, tag="raccbf",
                                       name="racc_bf")
                    stg_q = [None] * 8
                    for bb in range(9):
                        if bb < 8:
                            # scores + exp for burst bb
                            stg = stp.tile([P, 4 * MT], BF, tag="st",
                                           name="stg")
                            stg_q[bb] = stg
                            for half in range(2):
                                pt = psA.tile([P, 2, MT], FP, tag="pt",
                                              name="pt")
                                for rr in range(2):
                                    rg = 2 * half + rr
                                    ksl = slice(rg * 1024 + bb * P,
                                                rg * 1024 + (bb + 1) * P)
                                    nc.tensor.matmul(
                                        pt[:, rr, :], kt[d][:, ksl],
                                        qr[d][:, msl],
                                        start=True, stop=True)
                                nc.scalar.activation(
                                    stg[:, half * 1024:(half + 1) * 1024],
                                    pt[:, :, :], _ACT.Exp)
                        if bb == 0 and pend_tail is not None:
                            pend_tail()
                            pend_tail = None
                        if bb == 1 and pend_yc is not None:
                            pend_yc()
                            pend_yc = None
                        if bb == 2 and pend_stats is not None:
                            pend_stats()
                            pend_stats = None
                        if bb >= 1:
                            # AV + rowsum for burst bb-1
                            b = bb - 1
                            stg = stg_q[b]
                            for rg in range(4):
                                j = rg * 8 + b
                                ssl = slice(rg * MT, (rg + 1) * MT)
                                for cc in range(2):
                                    nc.tensor.matmul(
                                        av[cc],
                                        vt[d][j][:, cc * P:(cc + 1) * P],
                                        stg[:, ssl],
                                        start=(b == 0 and rg == 0),
                                        stop=(b == 7 and rg == 3))
                            t1 = rsp.tile([P, 2 * MT], BF, tag="t1",
                                          name="t1")
                            nc.vector.tensor_add(t1[:], stg[:, 0:1024],
                                                 stg[:, 1024:2048])
                            t2 = rsp.tile([P, MT], BF, tag="t2", name="t2")
                            nc.vector.tensor_add(t2[:], t1[:, 0:MT],
                                                 t1[:, MT:2 * MT])
                            if b == 0:
                                nc.gpsimd.tensor_copy(racc[:], t2[:])
                            else:
                                nc.gpsimd.tensor_add(
                                    racc_bf[:] if b == 7 else racc[:],
                                    racc[:], t2[:])
                    pend_tail = make_tail(t, d, av, racc_bf)
                pend_yc = make_yc(t)
                pend_stats = make_stats(t)

            pend_tail()
            pend_yc()
            pend_stats()

            # ---- BN: collapse partials, AllReduce, normalize ----
            stats = p_small.tile([P, 4], FP, tag="stats", name="stats")
            for cc in range(2):
                nc.vector.reduce_sum(stats[:, cc:cc + 1], ssum[:, cc, :],
                                     axis=mybir.AxisListType.X)
                nc.vector.reduce_sum(stats[:, 2 + cc:3 + cc], ssq[:, cc, :],
                                     axis=mybir.AxisListType.X)
            cc_in = dram.tile([P, 4], FP, name="cc_in")
            cc_out = dram.tile([P, 4], FP, name="cc_out")
            nc.sync.dma_start(cc_in[:], stats[:])
            nc.gpsimd.collective_compute(
                "AllReduce", _ALU.add,
                replica_groups=[list(range(NCORES))],
                ins=[cc_in.opt()], outs=[cc_out.opt()])
            ar = p_small.tile([P, 4], FP, tag="ar", name="ar")
            nc.sync.dma_start(ar[:], cc_out[:])

            inv_n = 1.0 / BN_COUNT
            yo = yout.rearrange("(o p) m -> p o m", p=P)
            for cc in range(2):
                mean = p_small.tile([P, 1], FP, tag="bn", name="mean")
                ex2 = p_small.tile([P, 1], FP, tag="bn", name="ex2")
                var = p_small.tile([P, 1], FP, tag="bn", name="var")
                nc.vector.tensor_scalar_mul(mean[:], ar[:, cc:cc + 1], inv_n)
                nc.vector.tensor_scalar_mul(ex2[:], ar[:, 2 + cc:3 + cc],
                                            inv_n)
                nc.vector.tensor_tensor(var[:], mean[:], mean[:], _ALU.mult)
                nc.vector.tensor_sub(var[:], ex2[:], var[:])
                sd = p_small.tile([P, 1], FP, tag="bn", name="sd")
                nc.vector.tensor_scalar_add(var[:], var[:], BN_EPS)
                nc.scalar.activation(sd[:], var[:], _ACT.Sqrt)
                rstd = p_small.tile([P, 1], FP, tag="bn", name="rstd")
                nc.vector.reciprocal(rstd[:], sd[:])
                scale = p_small.tile([P, 1], FP, tag="bn", name="scale")
                nc.vector.tensor_tensor(scale[:], gb_sb[:, cc:cc + 1],
                                        rstd[:], _ALU.mult)
                shift = p_small.tile([P, 1], FP, tag="bn", name="shift")
                nc.vector.tensor_tensor(shift[:], mean[:], scale[:],
                                        _ALU.mult)
                nc.vector.tensor_sub(shift[:], gb_sb[:, 2 + cc:3 + cc],
                                     shift[:])
                for q in range(2):
                    qsl = slice(q * 1024, (q + 1) * 1024)
                    nc.vector.tensor_scalar(
                        out=y_acc[cc][:, qsl], in0=y_acc[cc][:, qsl],
                        scalar1=scale[:], scalar2=shift[:],
                        op0=_ALU.mult, op1=_ALU.add)
                    nc.sync.dma_start(yo[:, cc, qsl], y_acc[cc][:, qsl])

    nc.compile()
    return nc


def _get_program():
    global _PROGRAM
    if _PROGRAM is None:
        _PROGRAM = _build_program()
    return _PROGRAM


def _make_in_maps(inputs):
    BF_NP = mybir.dt.np(mybir.dt.bfloat16)
    f_p = np.ascontiguousarray(
        np.asarray(inputs["f_p"], np.float32).reshape(4, C, N))
    f_v = np.ascontiguousarray(
        np.asarray(inputs["f_v"], np.float32).reshape(4, C, N))

    def T(x):
        return np.ascontiguousarray(np.asarray(x, np.float32).T)

    w_out = np.asarray(inputs["w_out"], np.float32)
    bv_v = np.asarray(inputs["bv_v"], np.float32)
    bv_p = np.asarray(inputs["bv_p"], np.float32)
    # wv-bias terms of the cross contributions, folded into one vector.
    yb = w_out[:, 2 * C:3 * C] @ bv_v + w_out[:, 3 * C:] @ bv_p
    shared = {
        "wq0": T(inputs["wq_p"]), "wk0": T(inputs["wk_v"]),
        "wv0": T(inputs["wv_v"]),
        "wq1": T(inputs["wq_v"]), "wk1": T(inputs["wk_p"]),
        "wv1": T(inputs["wv_p"]),
        "wout": T(w_out[:, :2 * C]),
        "woutc": np.ascontiguousarray(T(w_out[:, 2 * C:]).astype(BF_NP)),
        "biasq": np.ascontiguousarray(np.stack(
            [np.tile(np.asarray(inputs[k], np.float32), 4)
             for k in ("bq_p", "bk_v", "bq_v", "bk_p")], axis=1)),
        "ybias": np.ascontiguousarray(np.stack([yb[:P], yb[P:]], axis=1)),
        "gb": np.ascontiguousarray(np.stack(
            [np.asarray(inputs["gamma"], np.float32)[:P],
             np.asarray(inputs["gamma"], np.float32)[P:],
             np.asarray(inputs["beta"], np.float32)[:P],
             np.asarray(inputs["beta"], np.float32)[P:]], axis=1)),
    }
    in_maps = []
    for core in range(NCORES):
        b, h = divmod(core, 2)
        # roll so this core's query half sits at columns [0, 2048); K/V use
        # the full (permuted) range -- softmax/AV are key-order-invariant.
        kv1 = np.ascontiguousarray(np.roll(f_p[b], -h * M, axis=1))
        kv0 = np.ascontiguousarray(np.roll(f_v[b], -h * M, axis=1))
        in_maps.append({"kv0": kv0, "kv1": kv1, **shared})
    return in_maps


def _assemble(results):
    out = np.empty((4, C, N), np.float32)
    for core in range(NCORES):
        b, h = divmod(core, 2)
        out[b][:, h * M:(h + 1) * M] = results[core]["y"]
    return out.reshape(4, C, 64, 64)


def _run(inputs, **kwargs):
    nc = _get_program()
    in_maps = _make_in_maps(inputs)
    res = bass_utils.run_bass_kernel_spmd(
        nc, in_maps, core_ids=list(range(NCORES)), **kwargs)
    return _assemble(res.results), res


def kernel(**inputs):
    out, _ = _run(inputs)
    return out
